# revision 36
# baseline (speedup 1.0000x reference)
"""Trainium2 Bass kernel for nn_MultiHeadAttention (B=2, S=2048, E=1024, H=16).

Sharding: 8 NeuronCores = data-parallel over the 2 batches x tensor-parallel
over the 16 heads in 4 groups of 4 heads (Wq/Wk/Wv split column-wise, Wo
row-wise).  Each core computes a full-[S, E] partial of its batch's output;
the host sums the 4 head-group partials per batch.

Per-core algorithm (fp16 matmul inputs, fp32 PSUM accumulation; fp8 is ruled
out: with random weights the attention output is itself a 1/sqrt(N_eff)-scale
average, so per-element quantization noise lands 1:1 on the output):
  Q.T/K.T[n,s] = (w chunk).T @ xT chunk    s-window pipelined behind the DMAs
  V[s,n]       = (xvT chunk).T @ wv chunk  v16 = [V_h|ones] even / [ones|V_h] odd
  S.T_h[k,q]   = K_h.T @ Q_h.T             row-packed head pairs (d=64)
  P.T          = exp(S.T/8) on ACT -> fp16
  [O.T;sums]   = v16_h.T @ P.T_h           O rows 0:64 even / 64:128 odd heads,
                                           rowsum broadcast on the other half
  O.Tn         = O.T * recip(sums)         recip half shifted via DMA
  out[m,:]     = sum_j (oT2 chunk).T @ wo2_j   fp32r, contraction 128 (head
                                               pairs packed on partitions)

Engines: PE is the bottleneck (~164us); exp runs entirely on ACT (~135us);
DVE does drains/normalize/stages; output-projection tiles and V-projection
tiles are interleaved into the score/PV stream so PE never idles.
"""

import numpy as np
from collections import deque
from contextlib import ExitStack

import concourse.bass as bass
import concourse.mybir as mybir
import concourse.tile as tile
from concourse.tile import ScopedClock
from concourse.bass_utils import run_bass_kernel_spmd

# ---------------------------------------------------------------------------
# Workarounds for the walrus build on this stack, which rejects more than ONE
# semaphore wait per instruction ("Too many sync wait commands").
# ---------------------------------------------------------------------------
_orig_commit_instruction = tile.TileContext._commit_instruction


def _commit_instruction(self, inst, lazy_reg_writes=True):
    si = getattr(inst, "sync_info", None)
    if si is not None and si.on_wait and len(si.on_wait) > 1:
        waits = list(si.on_wait)
        for w in waits[:-1]:
            nop = mybir.InstNoOp(
                name=self.nc.get_next_instruction_name(),
                ins=[], outs=[], engine=inst.engine,
            )
            nop.bass_nofuse = True
            nop.sync_info = mybir.SyncInfo(on_wait=[w], on_update=[])
            _orig_commit_instruction(self, nop, lazy_reg_writes=False)
        inst.sync_info = mybir.SyncInfo(
            on_wait=[waits[-1]], on_update=list(si.on_update or [])
        )
    return _orig_commit_instruction(self, inst, lazy_reg_writes)


def _drain_and_barrier(self, tick_clock, wait_clock):
    nc = self.nc
    drain_inst = nc.sync.drain()
    wait_clock.add_sem_waits(
        drain_inst.ins, ScopedClock({None: tick_clock.global_clock})
    )
    si = drain_inst.ins.sync_info
    waits = list(si.on_wait) if si and si.on_wait else []
    if len(waits) > 1:
        drain_inst.ins.sync_info = mybir.SyncInfo(
            on_wait=waits[:1], on_update=list(si.on_update or [])
        )
        for w in waits[1:]:
            extra = nc.sync.drain()
            esi = extra.ins.sync_info
            extra.ins.sync_info = mybir.SyncInfo(
                on_wait=[w],
                on_update=list(esi.on_update or []) if esi else [],
            )
    nc.all_engine_barrier()
    assert self.sems is not None
    popped = nc._tile_sem_poison_stack.pop()
    assert popped is self._sem_poison
    nc.clear_and_free_semaphores(list(self.sems.allocated().values()))
    nc.all_engine_barrier()


def _apply_tilefix():
    tile.TileContext._commit_instruction = _commit_instruction
    tile.TileContext._drain_and_barrier = _drain_and_barrier


_apply_tilefix()

# ---------------------------------------------------------------------------
# Problem constants (hardcoded)
# ---------------------------------------------------------------------------
B, S, E, H = 2, 2048, 1024, 16
HC, D = 4, 64              # heads per core, head dim
NCORES = 8
NE = E // 128              # 8  e-chunks
NQ = S // 512              # 4  q/s-windows
NK = S // 128              # 16 k-chunks
NM = S // 128              # 16 m-chunks

F32 = mybir.dt.float32
F32R = mybir.dt.float32r
FP16 = mybir.dt.float16
BF16 = mybir.dt.bfloat16
I16 = mybir.dt.int16

ALPHA = 0.125              # 1/sqrt(D) folded into the exp
LOG2E = 1.4426950408889634
SCH_A = ALPHA * LOG2E * 128.0      # Schraudolph exp: bf16 bits of e^(s/8)
SCH_B = 128.0 * 127.0 - 7.25


def build(ptbufs=6, ovbufs=2, sbufs=3, pv_lag=6, dve_ks=(3, 7, 11, 15),
          stage_eng="D" * 24 + "AD" * 4):
    nc = bass.Bass()
    xq = nc.dram_tensor("xq", [E, S], FP16, kind="ExternalInput")
    xk = nc.dram_tensor("xk", [E, S], FP16, kind="ExternalInput")
    xv = nc.dram_tensor("xv", [E, S], FP16, kind="ExternalInput")
    wq = nc.dram_tensor("wq", [E, 256], FP16, kind="ExternalInput")
    wk = nc.dram_tensor("wk", [E, 256], FP16, kind="ExternalInput")
    wv = nc.dram_tensor("wv", [E, 256], FP16, kind="ExternalInput")
    woT = nc.dram_tensor("woT", [256, E], F32R, kind="ExternalInput")
    out = nc.dram_tensor("out", [S, E], F32, kind="ExternalOutput")

    with tile.TileContext(nc) as tc, ExitStack() as ctx:
        consts = ctx.enter_context(tc.tile_pool(name="consts", bufs=1))
        wpool = ctx.enter_context(tc.tile_pool(name="w", bufs=1))
        actpool = ctx.enter_context(tc.tile_pool(name="acts", bufs=1))
        xpool = ctx.enter_context(tc.tile_pool(name="x", bufs=1))
        ppool = ctx.enter_context(tc.tile_pool(name="pT", bufs=ptbufs))
        p16pool = ctx.enter_context(tc.tile_pool(name="pT16", bufs=4))
        rpool = ctx.enter_context(tc.tile_pool(name="recip", bufs=2))
        opool = ctx.enter_context(tc.tile_pool(name="ostage", bufs=2))
        psS = ctx.enter_context(tc.tile_pool(name="psS", bufs=sbufs, space="PSUM"))
        psOV = ctx.enter_context(tc.tile_pool(name="psOV", bufs=ovbufs, space="PSUM"))

        # preload the exp table before the hot loop
        dummy = consts.tile([1, 8], F32)
        nc.vector.memset(dummy[:], 0.0)
        nc.scalar.activation(dummy[:], dummy[:], mybir.ActivationFunctionType.Exp)

        w_q = wpool.tile([128, NE, 256], FP16)
        w_k = wpool.tile([128, NE, 256], FP16)
        w_v = wpool.tile([128, NE, 256], FP16)
        wo2 = wpool.tile([128, 2, E], F32R)

        x_q = xpool.tile([128, NE, S], FP16)
        x_k = xpool.tile([128, NE, S], FP16)
        x_v = xpool.tile([128, NE, S], FP16)

        qT = actpool.tile([128, 2, S], FP16)    # [(2h x d), pair, s]
        kT = actpool.tile([128, 2, S], FP16)
        # v16[s%128, k, h, :]: even h [V|ones], odd h [ones|V]
        v16 = actpool.tile([128, NK, HC, 128], BF16)
        oT2 = actpool.tile([128, 2, S], F32R)   # [(h%2)*64+d, pair, s]

        v16h = v16.rearrange("p k (hh two) c -> p k hh two c", two=2)
        nc.gpsimd.memset(v16h[:, :, :, 0, 64:128], 1.0)
        nc.gpsimd.memset(v16h[:, :, :, 1, 0:64], 1.0)

        # ---- input DMA: K first, then Q, V; all s-window chunked ----
        def ldx(x_sb, x_dr, w, q):
            ws = slice(w * 512, (w + 1) * 512)
            q.dma_start(
                x_sb[:, :, ws],
                x_dr[:, ws].rearrange("(ec p) s -> p ec s", p=128))

        nc.sync.dma_start(w_k[:], wk.rearrange("(ec p) n -> p ec n", p=128))
        ldx(x_k, xk, 0, nc.sync)
        nc.sync.dma_start(w_q[:], wq.rearrange("(ec p) n -> p ec n", p=128))
        ldx(x_q, xq, 0, nc.sync)
        nc.scalar.dma_start(w_v[:], wv.rearrange("(ec p) n -> p ec n", p=128))
        ldx(x_v, xv, 0, nc.scalar)
        for w in range(1, NQ):
            ldx(x_k, xk, w, nc.sync)
            ldx(x_q, xq, w, nc.sync)
            ldx(x_v, xv, w, nc.scalar)
        nc.scalar.dma_start(wo2[:], woT.rearrange("(j p) e -> p j e", p=128))

        # ---- K/Q projections, emitted in quarter-window chunks so fill
        # work never blocks the score stream for long ----
        proj_ps = {}

        def proj_quarter(w_sb, x_sb, dst, w, quarter, nm):
            ws = slice(w * 512, (w + 1) * 512)
            key = (nm, w)
            if key not in proj_ps:
                proj_ps[key] = psS.tile([128, 1024], F32, tag="s",
                                        name=f"pj{nm}{w}")
            ps = proj_ps[key]
            nch, eh = quarter // 2, quarter % 2
            for e in range(eh * 4, eh * 4 + 4):
                nc.tensor.matmul(
                    ps[:, nch * 512:(nch + 1) * 512],
                    w_sb[:, e, nch * 128:(nch + 1) * 128],
                    x_sb[:, e, ws],
                    start=(e == 0), stop=(e == NE - 1),
                )
            if quarter == 3:
                nc.scalar.copy(
                    dst[:, :, ws],
                    ps[:].rearrange("p (nch s) -> p nch s", nch=2))
                del proj_ps[key]

        def proj_window(w_sb, x_sb, dst, w, nm):
            for quarter in range(4):
                proj_quarter(w_sb, x_sb, dst, w, quarter, nm)

        # window 0 of K/Q up front; windows 1-3 are JIT-woven into the
        # first attention unit's k-loop (fill schedule below)
        proj_window(w_k, x_k, kT, 0, "k")
        proj_window(w_q, x_q, qT, 0, "q")

        # ---- V projection tile (via psX; interleaved into the qc0 stream) --
        def v_proj_tile(m):
            ps = psS.tile([128, 512], F32, tag="s", name=f"vp{m}")
            for e in range(NE):
                nc.tensor.matmul(
                    ps[:, 0:256],
                    x_v[:, e, m * 128:(m + 1) * 128],
                    w_v[:, e, :],
                    start=(e == 0), stop=(e == NE - 1),
                )
            psh = ps[:, 0:256].rearrange("p (hh two c) -> p hh two c", two=2, c=64)
            nc.scalar.copy(v16h[:, m, :, 0, 0:64], psh[:, :, 0, :])
            nc.scalar.copy(v16h[:, m, :, 1, 64:128], psh[:, :, 1, :])

        pending_out = deque()

        def out_proj_tile(m):
            stage = opool.tile([128, E], F32)
            for jj in range(2):
                ps = psS.tile([128, 512], F32, tag="s", name=f"op{m}_{jj}")
                for j in range(2):
                    nc.tensor.matmul(
                        ps[:],
                        oT2[:, j, m * 128:(m + 1) * 128],
                        wo2[:, j, jj * 512:(jj + 1) * 512],
                        start=(j == 0), stop=(j == 1),
                    )
                if stage_eng[m * 2 + jj] == "A":
                    nc.scalar.copy(stage[:, jj * 512:(jj + 1) * 512], ps[:])
                else:
                    nc.vector.tensor_copy(stage[:, jj * 512:(jj + 1) * 512], ps[:])
                nc.sync.dma_start(
                    out[m * 128:(m + 1) * 128, jj * 512:(jj + 1) * 512],
                    stage[:, jj * 512:(jj + 1) * 512])

        # ---- steady state ----
        # v16 for the first k-chunks must exist before the first PVs
        v_proj_tile(0)
        v_proj_tile(1)

        # Fill schedules: K/Q projection quarter-windows and V-projection
        # tiles placed just-in-time vs their DMA arrivals and consumer
        # deadlines in the first two units' k-loops.
        def K(w, q):
            return ("k", w, q)

        def Q(w, q):
            return ("q", w, q)

        def V(m):
            return ("v", m)

        fill_sched = {
            (0, 0): {
                0: [V(2), V(3)], 1: [V(4), V(5)],
                2: [K(1, 0), K(1, 1)], 3: [K(1, 2), K(1, 3)],
                4: [V(6), V(7)], 5: [V(8), V(9)],
                6: [K(2, 0), K(2, 1)], 7: [K(2, 2), K(2, 3)],
                8: [V(10), V(11)], 9: [Q(1, 0), Q(1, 1)],
                10: [K(3, 0), K(3, 1)], 11: [K(3, 2), K(3, 3)],
                12: [V(12), V(13)], 13: [V(14), Q(1, 2)],
                14: [V(15), Q(1, 3)], 15: [Q(2, 0), Q(2, 1)],
            },
            (0, 1): {
                0: [Q(2, 2), Q(2, 3)], 1: [Q(3, 0), Q(3, 1)],
                2: [Q(3, 2), Q(3, 3)],
            },
        }

        def emit_fill_item(item):
            if item[0] == "v":
                v_proj_tile(item[1])
            elif item[0] == "k":
                proj_quarter(w_k, x_k, kT, item[1], item[2], "k")
            else:
                proj_quarter(w_q, x_q, qT, item[1], item[2], "q")

        for qc in range(NQ):
            qs = slice(qc * 512, (qc + 1) * 512)
            for pair in range(2):
                ps_ov = [psOV.tile([128, 512], F32, name=f"ov{qc}_{pair}_{i}",
                                   tag="ov") for i in range(2)]
                pv_queue = deque()

                def emit_pv():
                    for mm in pv_queue.popleft():
                        mm()

                unit_fills = fill_sched.get((qc, pair), {})
                for k in range(NK):
                    ks = slice(k * 128, (k + 1) * 128)
                    first, last = (k == 0), (k == NK - 1)
                    ps_s = psS.tile([128, 1024], F32, tag="s")
                    nc.tensor.matmul(ps_s[:, 0:512],
                                     kT[0:64, pair, ks], qT[0:64, pair, qs],
                                     start=True, stop=True)
                    nc.tensor.matmul(ps_s[:, 512:1024],
                                     kT[64:128, pair, ks], qT[64:128, pair, qs],
                                     start=True, stop=True)
                    if k in dve_ks:
                        pt16 = p16pool.tile([128, 1024], I16, tag="p16",
                                            name=f"p16_{qc}_{pair}_{k}")
                        nc.vector.tensor_scalar(
                            pt16[:], ps_s[:], SCH_A, SCH_B,
                            mybir.AluOpType.mult, mybir.AluOpType.add)
                        pT = pt16[:].bitcast(BF16)
                    else:
                        pt = ppool.tile([128, 1024], BF16, tag="p")
                        nc.scalar.activation(pt[:], ps_s[:],
                                             mybir.ActivationFunctionType.Exp,
                                             scale=ALPHA)
                        pT = pt[:]

                    def mk_pv(pT=pT, k=k, first=first, last=last):
                        for h2 in range(2):
                            h = pair * 2 + h2
                            nc.tensor.matmul(
                                ps_ov[h2][:],
                                v16[:, k, h, :],
                                pT[:, h2 * 512:(h2 + 1) * 512],
                                start=first, stop=last)
                    pv_queue.append([mk_pv])
                    if len(pv_queue) > pv_lag:
                        emit_pv()
                    for item in unit_fills.pop(k, []):
                        emit_fill_item(item)
                    if not unit_fills and k in (4, 12) and pending_out:
                        out_proj_tile(pending_out.popleft())
                while pv_queue:
                    emit_pv()

                # normalize: copy O half + recip of sums first (releases the
                # psOV bank before the shift-DMA latency), then multiply in
                # SBUF with the recip shifted to the O partitions
                for h2 in range(2):
                    oU = rpool.tile([128, 512], F32, tag="oU")
                    rt = rpool.tile([128, 512], F32, tag="rt")
                    rb = rpool.tile([128, 512], F32, tag="rb")
                    if h2 == 0:  # O rows 0:64, sums broadcast 64:128
                        nc.vector.tensor_copy(oU[0:64, :], ps_ov[0][0:64, :])
                        nc.vector.reciprocal(rt[64:128, :], ps_ov[0][64:128, :])
                        nc.sync.dma_start(rb[0:64, :], rt[64:128, :])
                        nc.vector.tensor_tensor(
                            oT2[0:64, pair, qs], oU[0:64, :],
                            rb[0:64, :], mybir.AluOpType.mult)
                    else:        # O rows 64:128, sums broadcast 0:64
                        nc.vector.tensor_copy(oU[64:128, :], ps_ov[1][64:128, :])
                        nc.vector.reciprocal(rt[0:64, :], ps_ov[1][0:64, :])
                        nc.sync.dma_start(rb[64:128, :], rt[0:64, :])
                        nc.vector.tensor_tensor(
                            oT2[64:128, pair, qs], oU[64:128, :],
                            rb[64:128, :], mybir.AluOpType.mult)

            pending_out.extend(range(qc * 4, qc * 4 + 4))

        while pending_out:
            out_proj_tile(pending_out.popleft())

    return nc


_NC_CACHE = {}


def _get_nc():
    if "nc" not in _NC_CACHE:
        _NC_CACHE["nc"] = build()
    return _NC_CACHE["nc"]


def _shard_inputs(query, key, value, Wq, Wk, Wv, Wo):
    """Host-side sharding + fp16 layout prep: core c = (batch c//4, group c%4)."""
    f16 = np.float16
    xT = []
    for b in range(B):
        xT.append((
            np.ascontiguousarray(query[b].T).astype(f16),
            np.ascontiguousarray(key[b].T).astype(f16),
            np.ascontiguousarray(value[b].T).astype(f16),
        ))
    wT = []
    for g in range(4):
        gc = slice(g * 256, (g + 1) * 256)
        wT.append((
            np.ascontiguousarray(Wq[gc].T).astype(f16),
            np.ascontiguousarray(Wk[gc].T).astype(f16),
            np.ascontiguousarray(Wv[gc].T).astype(f16),
            np.ascontiguousarray(Wo[:, gc].T).astype(np.float32),
        ))
    in_maps = []
    for c in range(NCORES):
        b, g = c // 4, c % 4
        qTa, kTa, vTa = xT[b]
        wq_, wk_, wv_, wo_ = wT[g]
        in_maps.append({
            "xq": qTa, "xk": kTa, "xv": vTa,
            "wq": wq_, "wk": wk_, "wv": wv_, "woT": wo_,
        })
    return in_maps


def kernel(query, key, value, Wq, Wk, Wv, Wo):
    query = np.asarray(query, dtype=np.float32)
    key = np.asarray(key, dtype=np.float32)
    value = np.asarray(value, dtype=np.float32)
    Wq = np.asarray(Wq, dtype=np.float32)
    Wk = np.asarray(Wk, dtype=np.float32)
    Wv = np.asarray(Wv, dtype=np.float32)
    Wo = np.asarray(Wo, dtype=np.float32)

    nc = _get_nc()
    in_maps = _shard_inputs(query, key, value, Wq, Wk, Wv, Wo)
    res = run_bass_kernel_spmd(nc, in_maps, core_ids=list(range(NCORES)))

    out = np.zeros((B, S, E), dtype=np.float32)
    for c in range(NCORES):
        out[c // 4] += res.results[c]["out"]
    return out


# revision 51
# speedup vs baseline: 1.0010x; 1.0010x over previous
"""Trainium2 Bass kernel for nn_MultiHeadAttention (B=2, S=2048, E=1024, H=16).

Sharding: 8 NeuronCores = data-parallel over the 2 batches x tensor-parallel
over the 16 heads in 4 groups of 4 heads (Wq/Wk/Wv split column-wise, Wo
row-wise).  Each core computes a full-[S, E] partial of its batch's output;
the host sums the 4 head-group partials per batch.

Per-core algorithm (fp16 matmul inputs, fp32 PSUM accumulation; fp8 is ruled
out: with random weights the attention output is itself a 1/sqrt(N_eff)-scale
average, so per-element quantization noise lands 1:1 on the output):
  Q.T/K.T[n,s] = (w chunk).T @ xT chunk    s-window pipelined behind the DMAs
  V[s,n]       = (xvT chunk).T @ wv chunk  v16 = [V_h|ones] even / [ones|V_h] odd
  S.T_h[k,q]   = K_h.T @ Q_h.T             row-packed head pairs (d=64)
  P.T          = exp(S.T/8) on ACT -> fp16
  [O.T;sums]   = v16_h.T @ P.T_h           O rows 0:64 even / 64:128 odd heads,
                                           rowsum broadcast on the other half
  O.Tn         = O.T * recip(sums)         recip half shifted via DMA
  out[m,:]     = sum_j (oT2 chunk).T @ wo2_j   fp32r, contraction 128 (head
                                               pairs packed on partitions)

Engines: PE is the bottleneck (~164us); exp runs entirely on ACT (~135us);
DVE does drains/normalize/stages; output-projection tiles and V-projection
tiles are interleaved into the score/PV stream so PE never idles.
"""

import numpy as np
from collections import deque
from contextlib import ExitStack

import concourse.bass as bass
import concourse.mybir as mybir
import concourse.tile as tile
from concourse.tile import ScopedClock
from concourse.bass_utils import run_bass_kernel_spmd

# ---------------------------------------------------------------------------
# Workarounds for the walrus build on this stack, which rejects more than ONE
# semaphore wait per instruction ("Too many sync wait commands").
# ---------------------------------------------------------------------------
_orig_commit_instruction = tile.TileContext._commit_instruction


def _commit_instruction(self, inst, lazy_reg_writes=True):
    si = getattr(inst, "sync_info", None)
    if si is not None and si.on_wait and len(si.on_wait) > 1:
        waits = list(si.on_wait)
        for w in waits[:-1]:
            nop = mybir.InstNoOp(
                name=self.nc.get_next_instruction_name(),
                ins=[], outs=[], engine=inst.engine,
            )
            nop.bass_nofuse = True
            nop.sync_info = mybir.SyncInfo(on_wait=[w], on_update=[])
            _orig_commit_instruction(self, nop, lazy_reg_writes=False)
        inst.sync_info = mybir.SyncInfo(
            on_wait=[waits[-1]], on_update=list(si.on_update or [])
        )
    return _orig_commit_instruction(self, inst, lazy_reg_writes)


def _drain_and_barrier(self, tick_clock, wait_clock):
    nc = self.nc
    drain_inst = nc.sync.drain()
    wait_clock.add_sem_waits(
        drain_inst.ins, ScopedClock({None: tick_clock.global_clock})
    )
    si = drain_inst.ins.sync_info
    waits = list(si.on_wait) if si and si.on_wait else []
    if len(waits) > 1:
        drain_inst.ins.sync_info = mybir.SyncInfo(
            on_wait=waits[:1], on_update=list(si.on_update or [])
        )
        for w in waits[1:]:
            extra = nc.sync.drain()
            esi = extra.ins.sync_info
            extra.ins.sync_info = mybir.SyncInfo(
                on_wait=[w],
                on_update=list(esi.on_update or []) if esi else [],
            )
    nc.all_engine_barrier()
    assert self.sems is not None
    popped = nc._tile_sem_poison_stack.pop()
    assert popped is self._sem_poison
    nc.clear_and_free_semaphores(list(self.sems.allocated().values()))
    nc.all_engine_barrier()


def _apply_tilefix():
    tile.TileContext._commit_instruction = _commit_instruction
    tile.TileContext._drain_and_barrier = _drain_and_barrier


_apply_tilefix()

# ---------------------------------------------------------------------------
# Problem constants (hardcoded)
# ---------------------------------------------------------------------------
B, S, E, H = 2, 2048, 1024, 16
HC, D = 4, 64              # heads per core, head dim
NCORES = 8
NE = E // 128              # 8  e-chunks
NQ = S // 512              # 4  q/s-windows
NK = S // 128              # 16 k-chunks
NM = S // 128              # 16 m-chunks

F32 = mybir.dt.float32
F32R = mybir.dt.float32r
FP16 = mybir.dt.float16
BF16 = mybir.dt.bfloat16
I16 = mybir.dt.int16

ALPHA = 0.125              # 1/sqrt(D) folded into the exp
LOG2E = 1.4426950408889634
SCH_A = ALPHA * LOG2E * 128.0      # Schraudolph exp: bf16 bits of e^(s/8)
SCH_B = 128.0 * 127.0 - 7.25


def build(ptbufs=6, ovbufs=2, sbufs=3, pv_lag=6, dve_ks=(3, 7, 11, 15),
          stage_eng="D" * 24 + "AD" * 4):
    nc = bass.Bass()
    xq = nc.dram_tensor("xq", [E, S], FP16, kind="ExternalInput")
    xk = nc.dram_tensor("xk", [E, S], FP16, kind="ExternalInput")
    xv = nc.dram_tensor("xv", [E, S], FP16, kind="ExternalInput")
    wq = nc.dram_tensor("wq", [E, 256], FP16, kind="ExternalInput")
    wk = nc.dram_tensor("wk", [E, 256], FP16, kind="ExternalInput")
    wv = nc.dram_tensor("wv", [E, 256], FP16, kind="ExternalInput")
    woT = nc.dram_tensor("woT", [256, E], F32R, kind="ExternalInput")
    out = nc.dram_tensor("out", [S, E], F32, kind="ExternalOutput")

    with tile.TileContext(nc) as tc, ExitStack() as ctx:
        consts = ctx.enter_context(tc.tile_pool(name="consts", bufs=1))
        wpool = ctx.enter_context(tc.tile_pool(name="w", bufs=1))
        actpool = ctx.enter_context(tc.tile_pool(name="acts", bufs=1))
        xpool = ctx.enter_context(tc.tile_pool(name="x", bufs=1))
        ppool = ctx.enter_context(tc.tile_pool(name="pT", bufs=ptbufs))
        p16pool = ctx.enter_context(tc.tile_pool(name="pT16", bufs=4))
        rpool = ctx.enter_context(tc.tile_pool(name="recip", bufs=2))
        opool = ctx.enter_context(tc.tile_pool(name="ostage", bufs=2))
        psS = ctx.enter_context(tc.tile_pool(name="psS", bufs=sbufs, space="PSUM"))
        psOV = ctx.enter_context(tc.tile_pool(name="psOV", bufs=ovbufs, space="PSUM"))

        # preload the exp table before the hot loop
        dummy = consts.tile([1, 8], F32)
        nc.vector.memset(dummy[:], 0.0)
        nc.scalar.activation(dummy[:], dummy[:], mybir.ActivationFunctionType.Exp)

        w_q = wpool.tile([128, NE, 256], FP16)
        w_k = wpool.tile([128, NE, 256], FP16)
        w_v = wpool.tile([128, NE, 256], FP16)
        wo2 = wpool.tile([128, 2, E], F32R)

        x_q = xpool.tile([128, NE, S], FP16)
        x_k = xpool.tile([128, NE, S], FP16)
        x_v = xpool.tile([128, NE, S], FP16)

        qT = actpool.tile([128, 2, S], FP16)    # [(2h x d), pair, s]
        kT = actpool.tile([128, 2, S], FP16)
        # v16[s%128, k, h, :]: even h [V|ones], odd h [ones|V]
        v16 = actpool.tile([128, NK, HC, 128], BF16)
        oT2 = actpool.tile([128, 2, S], F32R)   # [(h%2)*64+d, pair, s]

        v16h = v16.rearrange("p k (hh two) c -> p k hh two c", two=2)
        nc.gpsimd.memset(v16h[:, :, :, 0, 64:128], 1.0)
        nc.gpsimd.memset(v16h[:, :, :, 1, 0:64], 1.0)

        # ---- input DMA: K first, then Q, V; all s-window chunked ----
        def ldx(x_sb, x_dr, w, q):
            ws = slice(w * 512, (w + 1) * 512)
            q.dma_start(
                x_sb[:, :, ws],
                x_dr[:, ws].rearrange("(ec p) s -> p ec s", p=128))

        ldx(x_k, xk, 0, nc.sync)
        nc.sync.dma_start(w_k[:], wk.rearrange("(ec p) n -> p ec n", p=128))
        ldx(x_q, xq, 0, nc.sync)
        nc.sync.dma_start(w_q[:], wq.rearrange("(ec p) n -> p ec n", p=128))
        nc.scalar.dma_start(w_v[:], wv.rearrange("(ec p) n -> p ec n", p=128))
        ldx(x_v, xv, 0, nc.scalar)
        def ldx2(x_sb, x_dr, w0, q):
            ws = slice(w0 * 512, (w0 + 2) * 512)
            q.dma_start(
                x_sb[:, :, ws],
                x_dr[:, ws].rearrange("(ec p) s -> p ec s", p=128))

        ldx(x_k, xk, 1, nc.sync)
        ldx(x_q, xq, 1, nc.sync)
        ldx(x_v, xv, 1, nc.scalar)
        ldx2(x_k, xk, 2, nc.sync)
        ldx2(x_q, xq, 2, nc.sync)
        ldx2(x_v, xv, 2, nc.scalar)
        nc.scalar.dma_start(wo2[:], woT.rearrange("(j p) e -> p j e", p=128))

        # ---- K/Q projections, emitted in quarter-window chunks so fill
        # work never blocks the score stream for long ----
        proj_ps = {}

        def proj_quarter(w_sb, x_sb, dst, w, quarter, nm):
            ws = slice(w * 512, (w + 1) * 512)
            key = (nm, w)
            if key not in proj_ps:
                proj_ps[key] = psS.tile([128, 1024], F32, tag="s",
                                        name=f"pj{nm}{w}")
            ps = proj_ps[key]
            nch, eh = quarter // 2, quarter % 2
            for e in range(eh * 4, eh * 4 + 4):
                nc.tensor.matmul(
                    ps[:, nch * 512:(nch + 1) * 512],
                    w_sb[:, e, nch * 128:(nch + 1) * 128],
                    x_sb[:, e, ws],
                    start=(e == 0), stop=(e == NE - 1),
                )
            if quarter == 3:
                nc.scalar.copy(
                    dst[:, :, ws],
                    ps[:].rearrange("p (nch s) -> p nch s", nch=2))
                del proj_ps[key]

        def proj_window(w_sb, x_sb, dst, w, nm):
            for quarter in range(4):
                proj_quarter(w_sb, x_sb, dst, w, quarter, nm)

        # window 0 of K/Q up front; windows 1-3 are JIT-woven into the
        # first attention unit's k-loop (fill schedule below)
        proj_window(w_k, x_k, kT, 0, "k")
        proj_window(w_q, x_q, qT, 0, "q")

        # ---- V projection tile (via psX; interleaved into the qc0 stream) --
        def v_proj_tile(m):
            ps = psS.tile([128, 512], F32, tag="s", name=f"vp{m}")
            for e in range(NE):
                nc.tensor.matmul(
                    ps[:, 0:256],
                    x_v[:, e, m * 128:(m + 1) * 128],
                    w_v[:, e, :],
                    start=(e == 0), stop=(e == NE - 1),
                )
            psh = ps[:, 0:256].rearrange("p (hh two c) -> p hh two c", two=2, c=64)
            nc.scalar.copy(v16h[:, m, :, 0, 0:64], psh[:, :, 0, :])
            nc.scalar.copy(v16h[:, m, :, 1, 64:128], psh[:, :, 1, :])

        pending_out = deque()

        def out_proj_tile(m):
            stage = opool.tile([128, E], F32)
            for jj in range(2):
                ps = psS.tile([128, 512], F32, tag="s", name=f"op{m}_{jj}")
                for j in range(2):
                    nc.tensor.matmul(
                        ps[:],
                        oT2[:, j, m * 128:(m + 1) * 128],
                        wo2[:, j, jj * 512:(jj + 1) * 512],
                        start=(j == 0), stop=(j == 1),
                    )
                if stage_eng[m * 2 + jj] == "A":
                    nc.scalar.copy(stage[:, jj * 512:(jj + 1) * 512], ps[:])
                else:
                    nc.vector.tensor_copy(stage[:, jj * 512:(jj + 1) * 512], ps[:])
                nc.sync.dma_start(
                    out[m * 128:(m + 1) * 128, jj * 512:(jj + 1) * 512],
                    stage[:, jj * 512:(jj + 1) * 512])

        # ---- steady state ----
        # v16 for the first k-chunks must exist before the first PVs
        v_proj_tile(0)
        v_proj_tile(1)

        # Fill schedules: K/Q projection quarter-windows and V-projection
        # tiles placed just-in-time vs their DMA arrivals and consumer
        # deadlines in the first two units' k-loops.
        def K(w, q):
            return ("k", w, q)

        def Q(w, q):
            return ("q", w, q)

        def V(m):
            return ("v", m)

        fill_sched = {
            (0, 0): {
                0: [V(2), V(3)], 1: [V(4), V(5), K(1, 0), K(1, 1)],
                2: [K(1, 2), K(1, 3)],
                4: [V(6), V(7)], 5: [V(8), V(9)],
                6: [K(2, 0), K(2, 1)], 7: [K(2, 2), K(2, 3)],
                8: [V(10), V(11)], 9: [Q(1, 0), Q(1, 1)],
                10: [K(3, 0), K(3, 1)], 11: [K(3, 2), K(3, 3)],
                12: [V(12), V(13)], 13: [V(14), Q(1, 2)],
                14: [V(15), Q(1, 3)], 15: [Q(2, 0), Q(2, 1)],
            },
            (0, 1): {
                0: [Q(2, 2), Q(2, 3)], 1: [Q(3, 0), Q(3, 1)],
                2: [Q(3, 2), Q(3, 3)],
            },
        }

        def emit_fill_item(item):
            if item[0] == "v":
                v_proj_tile(item[1])
            elif item[0] == "k":
                proj_quarter(w_k, x_k, kT, item[1], item[2], "k")
            else:
                proj_quarter(w_q, x_q, qT, item[1], item[2], "q")

        for qc in range(NQ):
            qs = slice(qc * 512, (qc + 1) * 512)
            for pair in range(2):
                last_unit = (qc == NQ - 1 and pair == 1)
                unit_dve_ks = (3, 7, 11) if last_unit else dve_ks
                unit_lag = pv_lag
                ps_ov = [psOV.tile([128, 512], F32, name=f"ov{qc}_{pair}_{i}",
                                   tag="ov") for i in range(2)]
                pv_queue = deque()

                def emit_pv():
                    for mm in pv_queue.popleft():
                        mm()

                unit_fills = fill_sched.get((qc, pair), {})
                for k in range(NK):
                    ks = slice(k * 128, (k + 1) * 128)
                    first, last = (k == 0), (k == NK - 1)
                    ps_s = psS.tile([128, 1024], F32, tag="s")
                    nc.tensor.matmul(ps_s[:, 0:512],
                                     kT[0:64, pair, ks], qT[0:64, pair, qs],
                                     start=True, stop=True)
                    nc.tensor.matmul(ps_s[:, 512:1024],
                                     kT[64:128, pair, ks], qT[64:128, pair, qs],
                                     start=True, stop=True)
                    if k in unit_dve_ks:
                        pt16 = p16pool.tile([128, 1024], I16, tag="p16",
                                            name=f"p16_{qc}_{pair}_{k}")
                        nc.vector.tensor_scalar(
                            pt16[:], ps_s[:], SCH_A, SCH_B,
                            mybir.AluOpType.mult, mybir.AluOpType.add)
                        pT = pt16[:].bitcast(BF16)
                    else:
                        pt = ppool.tile([128, 1024], BF16, tag="p")
                        nc.scalar.activation(pt[:], ps_s[:],
                                             mybir.ActivationFunctionType.Exp,
                                             scale=ALPHA)
                        pT = pt[:]

                    def mk_pv(pT=pT, k=k, first=first, last=last):
                        for h2 in range(2):
                            h = pair * 2 + h2
                            nc.tensor.matmul(
                                ps_ov[h2][:],
                                v16[:, k, h, :],
                                pT[:, h2 * 512:(h2 + 1) * 512],
                                start=first, stop=last)
                    pv_queue.append([mk_pv])
                    lag_now = max(2, unit_lag - max(0, k - 12))
                    while len(pv_queue) > lag_now:
                        emit_pv()
                    for item in unit_fills.pop(k, []):
                        emit_fill_item(item)
                    if not unit_fills and k in (4, 12) and pending_out:
                        out_proj_tile(pending_out.popleft())
                while pv_queue:
                    emit_pv()

                # normalize: copy O half + recip of sums first (releases the
                # psOV bank before the shift-DMA latency), then multiply in
                # SBUF with the recip shifted to the O partitions
                for h2 in range(2):
                    oU = rpool.tile([128, 512], F32, tag="oU")
                    rt = rpool.tile([128, 512], F32, tag="rt")
                    rb = rpool.tile([128, 512], F32, tag="rb")
                    if h2 == 0:  # O rows 0:64, sums broadcast 64:128
                        nc.vector.tensor_copy(oU[0:64, :], ps_ov[0][0:64, :])
                        nc.vector.reciprocal(rt[64:128, :], ps_ov[0][64:128, :])
                        nc.sync.dma_start(rb[0:64, :], rt[64:128, :])
                        nc.vector.tensor_tensor(
                            oT2[0:64, pair, qs], oU[0:64, :],
                            rb[0:64, :], mybir.AluOpType.mult)
                    else:        # O rows 64:128, sums broadcast 0:64
                        nc.vector.tensor_copy(oU[64:128, :], ps_ov[1][64:128, :])
                        nc.vector.reciprocal(rt[0:64, :], ps_ov[1][0:64, :])
                        nc.sync.dma_start(rb[64:128, :], rt[0:64, :])
                        nc.vector.tensor_tensor(
                            oT2[64:128, pair, qs], oU[64:128, :],
                            rb[64:128, :], mybir.AluOpType.mult)

            pending_out.extend(range(qc * 4, qc * 4 + 4))

        while pending_out:
            out_proj_tile(pending_out.popleft())

    return nc


_NC_CACHE = {}


def _get_nc():
    if "nc" not in _NC_CACHE:
        _NC_CACHE["nc"] = build()
    return _NC_CACHE["nc"]


def _shard_inputs(query, key, value, Wq, Wk, Wv, Wo):
    """Host-side sharding + fp16 layout prep: core c = (batch c//4, group c%4)."""
    f16 = np.float16
    xT = []
    for b in range(B):
        xT.append((
            np.ascontiguousarray(query[b].T).astype(f16),
            np.ascontiguousarray(key[b].T).astype(f16),
            np.ascontiguousarray(value[b].T).astype(f16),
        ))
    wT = []
    for g in range(4):
        gc = slice(g * 256, (g + 1) * 256)
        wT.append((
            np.ascontiguousarray(Wq[gc].T).astype(f16),
            np.ascontiguousarray(Wk[gc].T).astype(f16),
            np.ascontiguousarray(Wv[gc].T).astype(f16),
            np.ascontiguousarray(Wo[:, gc].T).astype(np.float32),
        ))
    in_maps = []
    for c in range(NCORES):
        b, g = c // 4, c % 4
        qTa, kTa, vTa = xT[b]
        wq_, wk_, wv_, wo_ = wT[g]
        in_maps.append({
            "xq": qTa, "xk": kTa, "xv": vTa,
            "wq": wq_, "wk": wk_, "wv": wv_, "woT": wo_,
        })
    return in_maps


def kernel(query, key, value, Wq, Wk, Wv, Wo):
    query = np.asarray(query, dtype=np.float32)
    key = np.asarray(key, dtype=np.float32)
    value = np.asarray(value, dtype=np.float32)
    Wq = np.asarray(Wq, dtype=np.float32)
    Wk = np.asarray(Wk, dtype=np.float32)
    Wv = np.asarray(Wv, dtype=np.float32)
    Wo = np.asarray(Wo, dtype=np.float32)

    nc = _get_nc()
    in_maps = _shard_inputs(query, key, value, Wq, Wk, Wv, Wo)
    res = run_bass_kernel_spmd(nc, in_maps, core_ids=list(range(NCORES)))

    out = np.zeros((B, S, E), dtype=np.float32)
    for c in range(NCORES):
        out[c // 4] += res.results[c]["out"]
    return out
